# revision 101
# baseline (speedup 1.0000x reference)
"""BivectorRotarySelfAttention TRN2 kernel (bf16 pipeline).

Sharding: 8 cores = 4 batches x 2 head-halves. Each core computes one batch's
attention for 8 heads (2 kv heads) and a partial output projection; host sums
the two head-half partials per batch (bf16 partials, f32 sum).

Per-core dataflow (transposed layouts: features in partitions, seq in free):
  xT[ib]  = dma_transpose(x_bf16)                       16 x [128, L] bf16
  qT/kT/vT = W.T @ xT   (bf16 matmuls, PSUM f32, copied out as bf16)
  rope (per 512-half): psr = pmrot@qt (PE); t1 = psr*sin (DVE);
        t2 = qt*cos (Pool); qrt = t1+t2 (Pool)
  scores S^T[m,q]: 4 K=64 bf16 matmuls per 256-col chunk:
        psA = [S0 | c'*C0] (krt/kswap_h), psB = [S1 | C1] (krt/kswap)
        bs  = copy(psB) (Act), tp = psA*bs (DVE),
        raw = tp[:s] + tp[s:] (Pool), E = exp(alpha*raw + maskbias) (Act)
  causal: affine_select on diagonal blocks (Pool)
  outT[d,q] = vblk.T @ E; rowsums via ones-matmul; outtn = ps_o * rcp (DVE)
  y[l,:] = sum_h outtn_h.T @ Wo_h  (bf16 matmuls, bf16 out, f32 host sum)

Scheduling: engines execute in scheduled (program-priority) order, so the
head loop is software-pipelined by construction: head h's stripe loop emits
next head's q-projection/rope and the previous head's attnv c=1 as filler
units between score chunks; head 7 is filled with epilogue pre-accumulation
(heads 0..6 of the first Wo groups, parked to SBUF and finished via an
identity-matmul accumulate). DMA: weight copies first, ONE xbar switch,
then all 16 x-transposes back-to-back (copy<->transpose switches ~2.2us).
NOTE: Tile dependencies follow program order — a tile must be written
before any reader is emitted (head-0 kswap ordering).
"""
import sys
if '/opt/trn_rl_repo' not in sys.path:
    sys.path.insert(0, '/opt/trn_rl_repo')

import numpy as np
import ml_dtypes

import concourse.bass as bass
import concourse.mybir as mybir
import concourse.tile as tile
from concourse import bacc
from concourse.bass_utils import run_bass_kernel_spmd

F32 = mybir.dt.float32
BF16 = mybir.dt.bfloat16

B, L, D, H, HKV = 4, 1024, 2048, 16, 4
HD = D // H            # 128
HD2 = HD // 2          # 64
NH = 8                 # heads per core
NKV = 2                # kv heads per core
NB = L // 128          # 8 blocks of 128
AluOp = mybir.AluOpType
Act = mybir.ActivationFunctionType

_CACHED = {}


def _chunks_for_stripe(mb):
    """Q-column chunks [(qs, qe)] covering [128*mb, 1024), split at 256-multiples."""
    q0 = 128 * mb
    out = []
    while q0 < L:
        qe = min(L, (q0 // 256 + 1) * 256)
        out.append((q0, qe))
        q0 = qe
    return out


# packed E-tile column offsets: region for stripe mb starts at _EOFF[mb]
_EOFF = [0]
for _mb in range(NB):
    _EOFF.append(_EOFF[-1] + (L - 128 * _mb))
_ETOT = _EOFF[NB]          # 4608


def build_program():
    nc = bacc.Bacc("TRN2", target_bir_lowering=False, debug=False)

    # ---- dram params (per-core shapes) ----
    xh = nc.declare_dram_parameter("xh", [L, D], BF16, isOutput=False)
    wq = nc.declare_dram_parameter("wq", [128, NH, 16, 128], BF16, isOutput=False)
    wk = nc.declare_dram_parameter("wk", [128, NKV, 16, 128], BF16, isOutput=False)
    wv = nc.declare_dram_parameter("wv", [128, NKV, 16, 128], BF16, isOutput=False)
    wo = nc.declare_dram_parameter("wo", [128, NH, D], BF16, isOutput=False)
    cosq = nc.declare_dram_parameter("cosq", [128, NH, L], BF16, isOutput=False)
    sinq = nc.declare_dram_parameter("sinq", [128, NH, L], BF16, isOutput=False)
    cosk = nc.declare_dram_parameter("cosk", [128, NKV, L], BF16, isOutput=False)
    sink = nc.declare_dram_parameter("sink", [128, NKV, L], BF16, isOutput=False)
    maskb = nc.declare_dram_parameter("maskb", [128, NB], F32, isOutput=False)
    cprime = nc.declare_dram_parameter("cprime", [128, NH], F32, isOutput=False)
    alpha = nc.declare_dram_parameter("alpha", [128, NH], F32, isOutput=False)
    pmrot = nc.declare_dram_parameter("pmrot", [128, 128], BF16, isOutput=False)
    pmswap = nc.declare_dram_parameter("pmswap", [128, 128], BF16, isOutput=False)
    onesb = nc.declare_dram_parameter("onesb", [128, 128], BF16, isOutput=False)
    identb = nc.declare_dram_parameter("identb", [128, 128], BF16, isOutput=False)
    y = nc.declare_dram_parameter("y", [L, D], BF16, isOutput=True)

    with tile.TileContext(nc) as tc:
        with (
            tc.tile_pool(name="persist", bufs=1) as pp,
            tc.tile_pool(name="psum", bufs=1, space="PSUM") as psp,
        ):
            # PSUM tags: "qp" [128,512] bufs=1 (1 bank) for q projections,
            # "sc" [128,512] bufs=7 (7 banks) for scores/attnv/vT/epilogue.
            def qp_tile():
                return psp.tile([128, 512], F32, tag="qp", bufs=1, name="qp_t")

            def sc_tile(w=512, dt_=F32):
                return psp.tile([128, w], dt_, tag="sc", bufs=7, name="sc_t")

            # --- DMA order: weight copies first, ONE xbar switch, then all
            # 16 transposes back-to-back (copy<->transpose switches cost ~2.2us)
            wk_t = pp.tile([128, NKV, 16, 128], BF16, tag="wk", name="wk_t")
            wv_t = pp.tile([128, NKV, 16, 128], BF16, tag="wv", name="wv_t")
            xt = [pp.tile([128, L], BF16, tag=f"xt{ib}", name=f"xt{ib}")
                  for ib in range(16)]
            nc.sync.dma_start(wk_t[:, 0], wk[:, 0])
            nc.sync.dma_start(wv_t[:, 0], wv[:, 0])
            for ib in range(16):
                nc.sync.dma_start_transpose(xt[ib][:], xh[:, ib * 128:(ib + 1) * 128])
            nc.sync.dma_start(wk_t[:, 1], wk[:, 1])
            nc.sync.dma_start(wv_t[:, 1], wv[:, 1])

            # small consts + k tables + head-0 tables next
            consts = {}
            for nm, src, dt_ in [("pmrot", pmrot, BF16), ("pmswap", pmswap, BF16),
                                 ("onesb", onesb, BF16), ("identb", identb, BF16),
                                 ("maskb", maskb, F32), ("cprime", cprime, F32),
                                 ("alpha", alpha, F32)]:
                t = pp.tile(list(src.shape), dt_, tag=nm, name=nm)
                nc.sync.dma_start(t[:], src[:])
                consts[nm] = t
            csl = pp.tile([128, NKV, L], BF16, tag="cosk", name="csl")
            snl = pp.tile([128, NKV, L], BF16, tag="sink", name="snl")
            nc.sync.dma_start(csl[:], cosk[:])
            nc.sync.dma_start(snl[:], sink[:])

            krt = [pp.tile([128, L], BF16, tag=f"krt{g}", name=f"krt{g}")
                   for g in range(NKV)]
            kswap = [pp.tile([128, L], BF16, tag=f"ksw{g}", name=f"ksw{g}")
                     for g in range(NKV)]
            vblk = [pp.tile([128, 128], BF16, tag=f"vb{i}", name=f"vb{i}")
                    for i in range(NKV * NB)]
            outtn = [pp.tile([128, L], BF16, tag=f"ot{h}", name=f"ot{h}")
                     for h in range(NH)]
            wo_t = [pp.tile([128, D], BF16, tag=f"wo{hb}", name=f"wo{hb}")
                    for hb in range(NH)]

            # ---------------- prologue: k/v proj pipelined via sc psum slots
            with (tc.tile_pool(name="pro", bufs=1) as ppro,
                  tc.tile_pool(name="hl", bufs=1) as ph):
                kt_s, vt_s = [], []
                projs = []
                for g in range(NKV):
                    projs.append((wk_t, g, kt_s, f"kt{g}"))
                    projs.append((wv_t, g, vt_s, f"vt{g}"))
                for w_t, g, outl, tg in projs:
                    pj = [sc_tile(), sc_tile()]
                    for ib in range(16):
                        for c in range(2):
                            nc.tensor.matmul(
                                pj[c][:],
                                w_t[:, g, ib, :],
                                xt[ib][:, c * 512:(c + 1) * 512],
                                start=(ib == 0), stop=(ib == 15))
                    ot = ppro.tile([128, L], BF16, tag=tg, name="projout")
                    if tg.startswith("kt"):
                        nc.scalar.copy(ot[:, 0:512], pj[0][:])
                        nc.scalar.copy(ot[:, 512:1024], pj[1][:])
                    else:
                        nc.vector.tensor_copy(ot[:, 0:512], pj[0][:])
                        nc.vector.tensor_copy(ot[:, 512:1024], pj[1][:])
                    outl.append(ot)

                # v transposes (fill PE while k copies/ropes progress)
                for g in range(NKV):
                    for mb in range(NB):
                        pv = sc_tile(128, BF16)
                        nc.tensor.transpose(pv[:], vt_s[g][:, mb * 128:(mb + 1) * 128],
                                            consts["identb"][:])
                        if mb % 2 == 0:
                            nc.vector.tensor_copy(vblk[g * NB + mb][:], pv[:])
                        else:
                            nc.scalar.copy(vblk[g * NB + mb][:], pv[:])

                # k rotate matmuls
                psrk = {}
                for g in range(NKV):
                    psrk[g] = [sc_tile(), sc_tile()]
                    for c in range(2):
                        nc.tensor.matmul(psrk[g][c][:], consts["pmrot"][:],
                                         kt_s[g][:, c * 512:(c + 1) * 512])
                # c0 halves for both groups first, so the pswk c0 matmuls
                # (emitted in the same order) don't wait on c1's Pool chain
                for c in range(2):
                    for g in range(NKV):
                        cs = slice(c * 512, (c + 1) * 512)
                        t1k = ppro.tile([128, 512], BF16, tag="rtmp", bufs=2,
                                        name="t1k")
                        t2k = ppro.tile([128, 512], BF16, tag="rtmp", bufs=2,
                                        name="t2k")
                        nc.vector.tensor_mul(t1k[:], psrk[g][c][:], snl[:, g, cs])
                        nc.gpsimd.tensor_mul(t2k[:], kt_s[g][:, cs], csl[:, g, cs])
                        nc.vector.tensor_add(krt[g][:, cs], t1k[:], t2k[:])

                # ---------------- head-pipeline helpers
                qs_state = {}

                def q_dma(h):
                    st = {}
                    st["wq"] = ph.tile([128, 16, 128], BF16, tag="wq_h", bufs=2,
                                       name="wq_t")
                    nc.sync.dma_start(st["wq"][:], wq[:, h, :, :])
                    st["cq"] = ph.tile([128, L], BF16, tag="cq", bufs=2, name="cq")
                    st["sq"] = ph.tile([128, L], BF16, tag="sq", bufs=2, name="sq")
                    nc.sync.dma_start(st["cq"][:], cosq[:, h, :])
                    nc.sync.dma_start(st["sq"][:], sinq[:, h, :])
                    qs_state[h] = st

                def q_finish(h):
                    st = qs_state[h]
                    nc.scalar.copy(st["qt"][:, 512:1024], st["psqt"][:])
                    st["ksw_h"] = ph.tile([128, L], BF16, tag="ksw_h", bufs=2,
                                          name="kswap_h")
                    nc.vector.tensor_scalar_mul(
                        st["ksw_h"][:], kswap[h // 4][:],
                        consts["cprime"][:, h:h + 1])

                def q_rope(h, c):
                    st = qs_state[h]
                    if c == 0:
                        st["qrt"] = ph.tile([128, L], BF16, tag="qrt", bufs=2,
                                            name="qrt")
                    cs = slice(c * 512, (c + 1) * 512)
                    psr = sc_tile()
                    nc.tensor.matmul(psr[:], consts["pmrot"][:], st["qt"][:, cs])
                    t1 = ph.tile([128, 512], BF16, tag="qtmp", bufs=2, name="t1")
                    t2 = ph.tile([128, 512], BF16, tag="qtmp", bufs=2, name="t2")
                    nc.vector.tensor_mul(t1[:], psr[:], st["sq"][:, cs])
                    nc.gpsimd.tensor_mul(t2[:], st["qt"][:, cs], st["cq"][:, cs])
                    nc.vector.tensor_add(st["qrt"][:, cs], t1[:], t2[:])

                def attnv_units(h, c):
                    """Closures: accumulation steps + rowsums + normalize."""
                    st = qs_state[h]
                    g = h // 4
                    mbs = [mb for mb in range(NB) if 128 * mb < 512 * (c + 1)]
                    box = {}

                    def mk_step(i, mb):
                        def step():
                            if i == 0:
                                box["ps_o"] = sc_tile()
                            etile = st["etile"]
                            os_ = max(512 * c, 128 * mb)
                            oe = 512 * (c + 1)
                            esl = etile[:, _EOFF[mb] + os_ - 128 * mb:
                                        _EOFF[mb] + oe - 128 * mb]
                            st_, sp = (i == 0), (i == len(mbs) - 1)
                            nc.tensor.matmul(
                                box["ps_o"][:, os_ - 512 * c:oe - 512 * c],
                                vblk[g * NB + mb][:], esl, start=st_, stop=sp)
                        return step

                    def rowsums():
                        etile = st["etile"]
                        ps_rs = sc_tile()
                        box["ps_rs"] = ps_rs
                        for i, mb in enumerate(mbs):
                            os_ = max(512 * c, 128 * mb)
                            oe = 512 * (c + 1)
                            esl = etile[:, _EOFF[mb] + os_ - 128 * mb:
                                        _EOFF[mb] + oe - 128 * mb]
                            nc.tensor.matmul(
                                ps_rs[:, os_ - 512 * c:oe - 512 * c],
                                consts["onesb"][:], esl,
                                start=(i == 0), stop=(i == len(mbs) - 1))

                    def fin():
                        rcp = ph.tile([128, 512], F32, tag="rcp", bufs=2,
                                      name="rcp")
                        nc.vector.reciprocal_approx_fast(rcp[:], box["ps_rs"][:])
                        nc.vector.tensor_mul(
                            outtn[h][:, c * 512:(c + 1) * 512],
                            box["ps_o"][:], rcp[:])

                    return ([mk_step(i, mb) for i, mb in enumerate(mbs)]
                            + [rowsums, fin])

                def attnv_half(h, c):
                    for u in attnv_units(h, c):
                        u()

                def qproj_units(h):
                    def mk(u):
                        def step():
                            q_proj_ib(h, u)
                        return step
                    return [mk(u) for u in range(32)]

                # ---- epilogue group machinery (also used as head-7 filler)
                egroups = [(lb, c, cc) for lb in range(NB) for c in range(2)
                           for cc in range(2)]
                epi_pre = {}     # group -> held psum tile (hh 0..6 accumulated)
                epi_part = {}    # group -> sbuf bf16 partial (hh 0..6)

                def psy_mm(psy, lb, c, cc, hh, st_, sp):
                    nc.tensor.matmul(
                        psy[:],
                        outtn[hh][:, lb * 128:(lb + 1) * 128],
                        wo_t[hh][:, c * 1024 + cc * 512:
                                 c * 1024 + (cc + 1) * 512],
                        start=st_, stop=sp)

                def epi_pre_units(grp):
                    def mk(hh):
                        def step():
                            if hh == 0:
                                epi_pre[grp] = sc_tile()
                            psy_mm(epi_pre[grp], *grp, hh, hh == 0, False)
                        return step
                    return [mk(hh) for hh in range(NH - 1)]

                def epi_part_units(grp, di):
                    box = {}

                    def mk(hh):
                        def step():
                            if hh == 0:
                                box["psy"] = sc_tile()
                            psy_mm(box["psy"], *grp, hh, hh == 0,
                                   hh == NH - 2)
                        return step

                    def cp():
                        pt = ph.tile([128, 512], BF16, tag="epart", bufs=8,
                                     name="epart")
                        epi_part[grp] = pt
                        if di % 2 == 0:
                            nc.vector.tensor_copy(pt[:], box["psy"][:])
                        else:
                            nc.scalar.copy(pt[:], box["psy"][:])
                    return [mk(hh) for hh in range(NH - 1)] + [cp]

                def q_proj_ib(h, u):
                    # u in [0, 32): c-half = u // 16, ib = u % 16
                    st = qs_state[h]
                    c, ib = u // 16, u % 16
                    if u == 0:
                        st["qt"] = ph.tile([128, L], BF16, tag="qt_s", bufs=2,
                                           name="qt_s")
                        st["psqt"] = qp_tile()
                    elif u == 16:
                        st["psqt"] = qp_tile()
                    nc.tensor.matmul(
                        st["psqt"][:],
                        st["wq"][:, ib, :],
                        xt[ib][:, c * 512:(c + 1) * 512],
                        start=(ib == 0), stop=(ib == 15))
                    if u == 15:
                        # issue the c0 copy immediately (on DVE: Act is the
                        # hot queue at head start); c1's qp WAR resolves sooner
                        nc.vector.tensor_copy(st["qt"][:, 0:512], st["psqt"][:])

                # ---------------- software-pipelined head loop
                q_dma(0)
                q_dma(1)
                # Head-0 qproj fills PE while the k-rope elementwise chain
                # produces krt; kswap matmuls then run stall-free.
                for u in range(32):
                    q_proj_ib(0, u)
                # kswap = partition-halves swap of krt (pmswap permutation mm).
                # Must be emitted BEFORE q_finish(0), which reads kswap[0] —
                # Tile dependencies follow program order.
                pswk = {g: [None, None] for g in range(NKV)}
                for c in range(2):
                    for g in range(NKV):
                        pswk[g][c] = sc_tile()
                        nc.tensor.matmul(pswk[g][c][:], consts["pmswap"][:],
                                         krt[g][:, c * 512:(c + 1) * 512])
                for g in range(NKV):
                    nc.scalar.copy(kswap[g][:, 0:512], pswk[g][0][:])
                    nc.scalar.copy(kswap[g][:, 512:1024], pswk[g][1][:])
                q_finish(0)
                q_rope(0, 0)
                q_rope(0, 1)

                for h in range(NH):
                    st = qs_state[h]
                    g = h // 4
                    if h < NH - 2:
                        q_dma(h + 2)
                    if h == 4:
                        for hb in range(NH):
                            nc.sync.dma_start(wo_t[hb][:], wo[:, hb, :])
                    st["etile"] = ph.tile([128, _ETOT], BF16, tag="esc", bufs=2,
                                          name="etile")
                    etile = st["etile"]
                    qrt = st["qrt"]
                    kswap_h = st["ksw_h"]
                    # PE filler units, popped between score chunks. The attnv
                    # units sit between the two qproj c-halves so the qt-half0
                    # copy (qp slot WAR) is hidden behind attnv matmuls.
                    fillers = []
                    av = attnv_units(h - 1, 1) if h > 0 else []
                    if h < NH - 1:
                        qp_u = qproj_units(h + 1)
                        fillers += qp_u[:16] + av + qp_u[16:]
                        fillers.append(lambda hh=h + 1: q_finish(hh))
                    else:
                        # last head: fill with epilogue pre-accumulation
                        fillers += av
                        for grp in egroups[:2]:
                            fillers += epi_pre_units(grp)
                        for di, grp in enumerate(egroups[2:10]):
                            fillers += epi_part_units(grp, di)
                    fi = [0]

                    def pop_fill(n):
                        while fi[0] < len(fillers) and n > 0:
                            fillers[fi[0]]()
                            fi[0] += 1
                            n -= 1

                    rawts = {}

                    def emit_exp(mb, rawts=rawts, etile=etile, h=h):
                        # exp deferred 2 stripes so Act's bs copies (which
                        # release score PSUM slots) aren't queued behind it.
                        # Per-head state bound via defaults (late-binding!).
                        w = L - 128 * mb
                        esl = etile[:, _EOFF[mb]:_EOFF[mb] + w]
                        nc.scalar.activation(esl, rawts.pop(mb)[:], Act.Exp,
                                             bias=consts["maskb"][:, mb:mb + 1],
                                             scale=consts["alpha"][:, h:h + 1])
                        # causal triangle on the diagonal 128 cols
                        nc.gpsimd.affine_select(
                            etile[:, _EOFF[mb]:_EOFF[mb] + 128],
                            etile[:, _EOFF[mb]:_EOFF[mb] + 128],
                            pattern=[[1, 128]], compare_op=AluOp.is_ge,
                            fill=0.0, base=0, channel_multiplier=-1)

                    st["emit_exp"] = emit_exp

                    if h == NH - 1:
                        # last head: attnv(h-1,1) fillers pop during stripe 0,
                        # so h-1's deferred exps must be emitted before them
                        qs_state[h - 1]["emit_exp"](4)
                        qs_state[h - 1]["emit_exp"](5)

                    # wide and narrow stripes interleaved so the elementwise
                    # consumers aren't front-loaded; stripes 4,5 defer their
                    # exps into the next head
                    SORDER = [0, 6, 1, 3, 2, 7, 4, 5]
                    ci = 0
                    for pos in range(NB):
                        mb = SORDER[pos]
                        kb = slice(mb * 128, (mb + 1) * 128)
                        w = L - 128 * mb
                        if pos >= 2:
                            emit_exp(SORDER[pos - 2])
                        if pos == 2 and 0 < h < NH - 1:
                            qs_state[h - 1]["emit_exp"](5)
                        rawt = ph.tile([128, w], BF16, tag="raw", bufs=4,
                                       name="rawt")
                        rawts[mb] = rawt
                        for (qs, qe) in _chunks_for_stripe(mb):
                            s = qe - qs
                            # psB first: its Act copy starts the consumer
                            # chain, so issue its matmuls before psA's
                            psB = sc_tile()
                            psA = sc_tile()
                            nc.tensor.matmul(psB[:, 0:s], krt[g][64:128, kb],
                                             qrt[64:128, qs:qe])
                            nc.tensor.matmul(psB[:, s:2 * s], kswap[g][64:128, kb],
                                             qrt[64:128, qs:qe])
                            nc.tensor.matmul(psA[:, 0:s], krt[g][0:64, kb],
                                             qrt[0:64, qs:qe])
                            nc.tensor.matmul(psA[:, s:2 * s], kswap_h[0:64, kb],
                                             qrt[0:64, qs:qe])
                            bs = ph.tile([128, 512], BF16, tag="bs", bufs=4,
                                         name="bs")
                            nc.scalar.copy(bs[:, 0:2 * s], psB[:, 0:2 * s])
                            tp = ph.tile([128, 512], BF16, tag="tprod", bufs=4,
                                         name="tp")
                            nc.vector.tensor_mul(tp[:, 0:2 * s], psA[:, 0:2 * s],
                                                 bs[:, 0:2 * s])
                            rsl = rawt[:, qs - 128 * mb:qe - 128 * mb]
                            if ci % 4 == 3:
                                # all-bf16 SBUF add runs in DVE 2x mode
                                nc.vector.tensor_add(
                                    rsl, tp[:, 0:s], tp[:, s:2 * s])
                            else:
                                nc.gpsimd.tensor_add(
                                    rsl, tp[:, 0:s], tp[:, s:2 * s])
                            ci += 1
                            if ci >= 2:
                                pop_fill(3 if ci < 6 else 2)
                        if pos == 1 and 0 < h < NH - 1:
                            # previous head's deferred exps, queued past this
                            # head's widest-stripe bs copies
                            qs_state[h - 1]["emit_exp"](4)
                        elif pos == 3:
                            pop_fill(len(fillers))
                            if h < NH - 1:
                                q_rope(h + 1, 0)
                        elif pos == 4:
                            if h < NH - 1:
                                q_rope(h + 1, 1)
                        elif pos == 7:
                            attnv_half(h, 0)
                    if h == NH - 1:
                        emit_exp(4)
                        emit_exp(5)

                # ------------ epilogue: Wo projection (finish)
                attnv_half(NH - 1, 1)

                yts = {}
                for grp in egroups:
                    lb, c, cc = grp
                    if (lb, c) not in yts:
                        yts[(lb, c)] = ph.tile([128, 1024], BF16, tag="ytile",
                                               bufs=2, name="yt")
                    yt = yts[(lb, c)]
                    if grp in epi_pre:
                        psy = epi_pre[grp]
                        psy_mm(psy, lb, c, cc, NH - 1, False, True)
                    elif grp in epi_part:
                        psy = sc_tile()
                        psy_mm(psy, lb, c, cc, NH - 1, True, False)
                        nc.tensor.matmul(psy[:], consts["identb"][:],
                                         epi_part[grp][:], start=False,
                                         stop=True)
                    else:
                        psy = sc_tile()
                        for hh in range(NH):
                            psy_mm(psy, lb, c, cc, hh, hh == 0, hh == NH - 1)
                    if cc == 0:
                        nc.vector.tensor_copy(yt[:, 0:512], psy[:])
                    else:
                        nc.scalar.copy(yt[:, 512:1024], psy[:])
                        nc.sync.dma_start(
                            y[lb * 128:(lb + 1) * 128, c * 1024:(c + 1) * 1024],
                            yt[:])

    nc.compile()
    return nc


def _host_prep(x, Wq, Wk, Wv, Wo, q_param, log_scale, cos, sin, mask):
    """Build the 8 per-core input maps."""
    x = np.asarray(x, np.float32)
    Wq = np.asarray(Wq, np.float32)
    Wk = np.asarray(Wk, np.float32)
    Wv = np.asarray(Wv, np.float32)
    Wo = np.asarray(Wo, np.float32)
    cos = np.asarray(cos, np.float32)[0]      # [L, H, 64]
    sin = np.asarray(sin, np.float32)[0]
    qp = np.asarray(q_param, np.float32).reshape(H)
    ls = np.asarray(log_scale, np.float32).reshape(H)
    mask = np.asarray(mask)

    p64 = np.arange(128) % 64

    PM = np.zeros((128, 128), np.float32)
    for dp in range(128):
        base, r = (dp // 64) * 64, dp % 64
        if r < 32:
            PM[base + r + 32, dp] = -1.0
        else:
            PM[base + r - 32, dp] = 1.0
    SW = np.zeros((128, 128), np.float32)
    for dp in range(128):
        SW[(dp + 64) % 128, dp] = 1.0
    PM = PM.astype(ml_dtypes.bfloat16)
    SW = SW.astype(ml_dtypes.bfloat16)
    ONES = np.ones((128, 128), ml_dtypes.bfloat16)
    IDENT = np.eye(128, dtype=ml_dtypes.bfloat16)

    in_maps = []
    for core in range(8):
        b, g2 = core // 2, core % 2
        heads = list(range(g2 * NH, (g2 + 1) * NH))
        kvs = list(range(g2 * NKV, (g2 + 1) * NKV))

        xh = x[b].astype(ml_dtypes.bfloat16)

        wq_c = Wq[:, g2 * NH * 128:(g2 + 1) * NH * 128]
        wk_c = Wk[:, g2 * NKV * 128:(g2 + 1) * NKV * 128]
        wv_c = Wv[:, g2 * NKV * 128:(g2 + 1) * NKV * 128]
        wo_c = Wo[g2 * NH * 128:(g2 + 1) * NH * 128, :]

        # wq: [128(part=K slice), NH, 16(ib), 128(dq)]
        wq_p = wq_c.reshape(16, 128, NH, 128).transpose(1, 2, 0, 3)
        wq_p = np.ascontiguousarray(wq_p).astype(ml_dtypes.bfloat16)
        wk_p = wk_c.reshape(16, 128, NKV, 128).transpose(1, 2, 0, 3)
        wk_p = np.ascontiguousarray(wk_p).astype(ml_dtypes.bfloat16)
        wv_p = wv_c.reshape(16, 128, NKV, 128).transpose(1, 2, 0, 3)
        wv_p = np.ascontiguousarray(wv_p).astype(ml_dtypes.bfloat16)
        wo_p = wo_c.reshape(NH, 128, D).transpose(1, 0, 2)
        wo_p = np.ascontiguousarray(wo_p).astype(ml_dtypes.bfloat16)

        cosq_p = np.ascontiguousarray(
            cos[:, heads, :][:, :, p64].transpose(2, 1, 0)).astype(ml_dtypes.bfloat16)
        sinq_p = np.ascontiguousarray(
            sin[:, heads, :][:, :, p64].transpose(2, 1, 0)).astype(ml_dtypes.bfloat16)
        cosk_p = np.ascontiguousarray(
            cos[:, kvs, :][:, :, p64].transpose(2, 1, 0)).astype(ml_dtypes.bfloat16)
        sink_p = np.ascontiguousarray(
            sin[:, kvs, :][:, :, p64].transpose(2, 1, 0)).astype(ml_dtypes.bfloat16)

        mb = np.where(mask[b].reshape(NB, 128).T.astype(bool), 0.0, -1e9)
        mb = mb.astype(np.float32)

        cpr = np.tile((-2.0 * np.tanh(qp[heads]))[None, :], (128, 1))
        alp = np.tile((np.exp(ls[heads]) / HD)[None, :], (128, 1))

        in_maps.append({
            "xh": xh,
            "wq": wq_p, "wk": wk_p, "wv": wv_p, "wo": wo_p,
            "cosq": cosq_p, "sinq": sinq_p, "cosk": cosk_p, "sink": sink_p,
            "maskb": mb, "cprime": cpr.astype(np.float32),
            "alpha": alp.astype(np.float32),
            "pmrot": PM, "pmswap": SW, "onesb": ONES, "identb": IDENT,
        })
    return in_maps


def kernel(**inputs):
    if "nc" not in _CACHED:
        _CACHED["nc"] = build_program()
    nc = _CACHED["nc"]
    in_maps = _host_prep(**inputs)
    res = run_bass_kernel_spmd(nc, in_maps, list(range(8))).results
    out = np.empty((B, L, D), np.float32)
    for b in range(B):
        out[b] = (res[2 * b]["y"].astype(np.float32)
                  + res[2 * b + 1]["y"].astype(np.float32))
    return out


# revision 102
# speedup vs baseline: 1.0086x; 1.0086x over previous
"""BivectorRotarySelfAttention TRN2 kernel (bf16 pipeline).

Sharding: 8 cores = 4 batches x 2 head-halves. Each core computes one batch's
attention for 8 heads (2 kv heads) and a partial output projection; host sums
the two head-half partials per batch (bf16 partials, f32 sum).

Per-core dataflow (transposed layouts: features in partitions, seq in free):
  xT[ib]  = dma_transpose(x_bf16)                       16 x [128, L] bf16
  qT/kT/vT = W.T @ xT   (bf16 matmuls, PSUM f32, copied out as bf16)
  rope (per 512-half): psr = pmrot@qt (PE); t1 = psr*sin (DVE);
        t2 = qt*cos (Pool); qrt = t1+t2 (Pool)
  scores S^T[m,q]: 4 K=64 bf16 matmuls per 256-col chunk:
        psA = [S0 | c'*C0] (krt/kswap_h), psB = [S1 | C1] (krt/kswap)
        bs  = copy(psB) (Act), tp = psA*bs (DVE),
        raw = tp[:s] + tp[s:] (Pool), E = exp(alpha*raw + maskbias) (Act)
  causal: affine_select on diagonal blocks (Pool)
  outT[d,q] = vblk.T @ E; rowsums via ones-matmul; outtn = ps_o * rcp (DVE)
  y[l,:] = sum_h outtn_h.T @ Wo_h  (bf16 matmuls, bf16 out, f32 host sum)

Scheduling: engines execute in scheduled (program-priority) order, so the
head loop is software-pipelined by construction: head h's stripe loop emits
next head's q-projection/rope and the previous head's attnv c=1 as filler
units between score chunks; head 7 is filled with epilogue pre-accumulation
(heads 0..6 of the first Wo groups, parked to SBUF and finished via an
identity-matmul accumulate). DMA: weight copies first, ONE xbar switch,
then all 16 x-transposes back-to-back (copy<->transpose switches ~2.2us).
NOTE: Tile dependencies follow program order — a tile must be written
before any reader is emitted (head-0 kswap ordering).
"""
import sys
if '/opt/trn_rl_repo' not in sys.path:
    sys.path.insert(0, '/opt/trn_rl_repo')

import numpy as np
import ml_dtypes

import concourse.bass as bass
import concourse.mybir as mybir
import concourse.tile as tile
from concourse import bacc
from concourse.bass_utils import run_bass_kernel_spmd

F32 = mybir.dt.float32
BF16 = mybir.dt.bfloat16

B, L, D, H, HKV = 4, 1024, 2048, 16, 4
HD = D // H            # 128
HD2 = HD // 2          # 64
NH = 8                 # heads per core
NKV = 2                # kv heads per core
NB = L // 128          # 8 blocks of 128
AluOp = mybir.AluOpType
Act = mybir.ActivationFunctionType

_CACHED = {}


def _chunks_for_stripe(mb):
    """Q-column chunks [(qs, qe)] covering [128*mb, 1024), split at 256-multiples."""
    q0 = 128 * mb
    out = []
    while q0 < L:
        qe = min(L, (q0 // 256 + 1) * 256)
        out.append((q0, qe))
        q0 = qe
    return out


# packed E-tile column offsets: region for stripe mb starts at _EOFF[mb]
_EOFF = [0]
for _mb in range(NB):
    _EOFF.append(_EOFF[-1] + (L - 128 * _mb))
_ETOT = _EOFF[NB]          # 4608


def build_program():
    nc = bacc.Bacc("TRN2", target_bir_lowering=False, debug=False)

    # ---- dram params (per-core shapes) ----
    xh = nc.declare_dram_parameter("xh", [L, D], BF16, isOutput=False)
    wq = nc.declare_dram_parameter("wq", [128, NH, 16, 128], BF16, isOutput=False)
    wk = nc.declare_dram_parameter("wk", [128, NKV, 16, 128], BF16, isOutput=False)
    wv = nc.declare_dram_parameter("wv", [128, NKV, 16, 128], BF16, isOutput=False)
    wo = nc.declare_dram_parameter("wo", [128, NH, D], BF16, isOutput=False)
    cosq = nc.declare_dram_parameter("cosq", [128, NH, L], BF16, isOutput=False)
    sinq = nc.declare_dram_parameter("sinq", [128, NH, L], BF16, isOutput=False)
    cosk = nc.declare_dram_parameter("cosk", [128, NKV, L], BF16, isOutput=False)
    sink = nc.declare_dram_parameter("sink", [128, NKV, L], BF16, isOutput=False)
    maskb = nc.declare_dram_parameter("maskb", [128, NB], F32, isOutput=False)
    cprime = nc.declare_dram_parameter("cprime", [128, NH], F32, isOutput=False)
    alpha = nc.declare_dram_parameter("alpha", [128, NH], F32, isOutput=False)
    pmrot = nc.declare_dram_parameter("pmrot", [128, 128], BF16, isOutput=False)
    pmswap = nc.declare_dram_parameter("pmswap", [128, 128], BF16, isOutput=False)
    onesb = nc.declare_dram_parameter("onesb", [128, 128], BF16, isOutput=False)
    identb = nc.declare_dram_parameter("identb", [128, 128], BF16, isOutput=False)
    y = nc.declare_dram_parameter("y", [L, D], BF16, isOutput=True)

    with tile.TileContext(nc) as tc:
        with (
            tc.tile_pool(name="persist", bufs=1) as pp,
            tc.tile_pool(name="psum", bufs=1, space="PSUM") as psp,
        ):
            # PSUM tags: "qp" [128,512] bufs=1 (1 bank) for q projections,
            # "sc" [128,512] bufs=7 (7 banks) for scores/attnv/vT/epilogue.
            def qp_tile():
                return psp.tile([128, 512], F32, tag="qp", bufs=1, name="qp_t")

            def sc_tile(w=512, dt_=F32):
                return psp.tile([128, w], dt_, tag="sc", bufs=7, name="sc_t")

            # --- DMA order: weight copies first, ONE xbar switch, then all
            # 16 transposes back-to-back (copy<->transpose switches cost ~2.2us)
            wk_t = pp.tile([128, NKV, 16, 128], BF16, tag="wk", name="wk_t")
            wv_t = pp.tile([128, NKV, 16, 128], BF16, tag="wv", name="wv_t")
            xt = [pp.tile([128, L], BF16, tag=f"xt{ib}", name=f"xt{ib}")
                  for ib in range(16)]
            nc.sync.dma_start(wk_t[:, 0], wk[:, 0])
            nc.sync.dma_start(wv_t[:, 0], wv[:, 0])
            for ib in range(16):
                nc.sync.dma_start_transpose(xt[ib][:], xh[:, ib * 128:(ib + 1) * 128])
            nc.sync.dma_start(wk_t[:, 1], wk[:, 1])
            nc.sync.dma_start(wv_t[:, 1], wv[:, 1])

            # small consts + k tables + head-0 tables next
            consts = {}
            for nm, src, dt_ in [("pmrot", pmrot, BF16), ("pmswap", pmswap, BF16),
                                 ("onesb", onesb, BF16), ("identb", identb, BF16),
                                 ("maskb", maskb, F32), ("cprime", cprime, F32),
                                 ("alpha", alpha, F32)]:
                t = pp.tile(list(src.shape), dt_, tag=nm, name=nm)
                nc.sync.dma_start(t[:], src[:])
                consts[nm] = t
            csl = pp.tile([128, NKV, L], BF16, tag="cosk", name="csl")
            snl = pp.tile([128, NKV, L], BF16, tag="sink", name="snl")
            nc.sync.dma_start(csl[:], cosk[:])
            nc.sync.dma_start(snl[:], sink[:])

            krt = [pp.tile([128, L], BF16, tag=f"krt{g}", name=f"krt{g}")
                   for g in range(NKV)]
            kswap = [pp.tile([128, L], BF16, tag=f"ksw{g}", name=f"ksw{g}")
                     for g in range(NKV)]
            vblk = [pp.tile([128, 128], BF16, tag=f"vb{i}", name=f"vb{i}")
                    for i in range(NKV * NB)]
            outtn = [pp.tile([128, L], BF16, tag=f"ot{h}", name=f"ot{h}")
                     for h in range(NH)]
            wo_t = [pp.tile([128, D], BF16, tag=f"wo{hb}", name=f"wo{hb}")
                    for hb in range(NH)]

            # ---------------- prologue: k/v proj pipelined via sc psum slots
            with (tc.tile_pool(name="pro", bufs=1) as ppro,
                  tc.tile_pool(name="hl", bufs=1) as ph):
                kt_s, vt_s = [], []
                projs = []
                for g in range(NKV):
                    projs.append((wk_t, g, kt_s, f"kt{g}"))
                    projs.append((wv_t, g, vt_s, f"vt{g}"))
                for w_t, g, outl, tg in projs:
                    pj = [sc_tile(), sc_tile()]
                    for ib in range(16):
                        for c in range(2):
                            nc.tensor.matmul(
                                pj[c][:],
                                w_t[:, g, ib, :],
                                xt[ib][:, c * 512:(c + 1) * 512],
                                start=(ib == 0), stop=(ib == 15))
                    ot = ppro.tile([128, L], BF16, tag=tg, name="projout")
                    if tg.startswith("kt"):
                        nc.scalar.copy(ot[:, 0:512], pj[0][:])
                        nc.scalar.copy(ot[:, 512:1024], pj[1][:])
                    else:
                        nc.vector.tensor_copy(ot[:, 0:512], pj[0][:])
                        nc.vector.tensor_copy(ot[:, 512:1024], pj[1][:])
                    outl.append(ot)

                # v transposes (fill PE while k copies/ropes progress)
                for g in range(NKV):
                    for mb in range(NB):
                        pv = sc_tile(128, BF16)
                        nc.tensor.transpose(pv[:], vt_s[g][:, mb * 128:(mb + 1) * 128],
                                            consts["identb"][:])
                        if mb % 2 == 0:
                            nc.vector.tensor_copy(vblk[g * NB + mb][:], pv[:])
                        else:
                            nc.scalar.copy(vblk[g * NB + mb][:], pv[:])

                # k rotate matmuls
                psrk = {}
                for g in range(NKV):
                    psrk[g] = [sc_tile(), sc_tile()]
                    for c in range(2):
                        nc.tensor.matmul(psrk[g][c][:], consts["pmrot"][:],
                                         kt_s[g][:, c * 512:(c + 1) * 512])
                # c0 halves for both groups first, so the pswk c0 matmuls
                # (emitted in the same order) don't wait on c1's Pool chain
                for c in range(2):
                    for g in range(NKV):
                        cs = slice(c * 512, (c + 1) * 512)
                        t1k = ppro.tile([128, 512], BF16, tag="rtmp", bufs=2,
                                        name="t1k")
                        t2k = ppro.tile([128, 512], BF16, tag="rtmp", bufs=2,
                                        name="t2k")
                        nc.vector.tensor_mul(t1k[:], psrk[g][c][:], snl[:, g, cs])
                        nc.gpsimd.tensor_mul(t2k[:], kt_s[g][:, cs], csl[:, g, cs])
                        nc.vector.tensor_add(krt[g][:, cs], t1k[:], t2k[:])

                # ---------------- head-pipeline helpers
                qs_state = {}

                def q_dma(h):
                    st = {}
                    st["wq"] = ph.tile([128, 16, 128], BF16, tag="wq_h", bufs=2,
                                       name="wq_t")
                    nc.sync.dma_start(st["wq"][:], wq[:, h, :, :])
                    st["cq"] = ph.tile([128, L], BF16, tag="cq", bufs=2, name="cq")
                    st["sq"] = ph.tile([128, L], BF16, tag="sq", bufs=2, name="sq")
                    nc.sync.dma_start(st["cq"][:], cosq[:, h, :])
                    nc.sync.dma_start(st["sq"][:], sinq[:, h, :])
                    qs_state[h] = st

                def q_finish(h):
                    st = qs_state[h]
                    nc.scalar.copy(st["qt"][:, 512:1024], st["psqt"][:])
                    st["ksw_h"] = ph.tile([128, L], BF16, tag="ksw_h", bufs=2,
                                          name="kswap_h")
                    nc.vector.tensor_scalar_mul(
                        st["ksw_h"][:], kswap[h // 4][:],
                        consts["cprime"][:, h:h + 1])

                def q_rope(h, c):
                    st = qs_state[h]
                    if c == 0:
                        st["qrt"] = ph.tile([128, L], BF16, tag="qrt", bufs=2,
                                            name="qrt")
                    cs = slice(c * 512, (c + 1) * 512)
                    psr = sc_tile()
                    nc.tensor.matmul(psr[:], consts["pmrot"][:], st["qt"][:, cs])
                    t1 = ph.tile([128, 512], BF16, tag="qtmp", bufs=2, name="t1")
                    t2 = ph.tile([128, 512], BF16, tag="qtmp", bufs=2, name="t2")
                    nc.vector.tensor_mul(t1[:], psr[:], st["sq"][:, cs])
                    nc.vector.tensor_mul(t2[:], st["qt"][:, cs], st["cq"][:, cs])
                    nc.vector.tensor_add(st["qrt"][:, cs], t1[:], t2[:])

                def attnv_units(h, c):
                    """Closures: accumulation steps + rowsums + normalize."""
                    st = qs_state[h]
                    g = h // 4
                    mbs = [mb for mb in range(NB) if 128 * mb < 512 * (c + 1)]
                    box = {}

                    def mk_step(i, mb):
                        def step():
                            if i == 0:
                                box["ps_o"] = sc_tile()
                            etile = st["etile"]
                            os_ = max(512 * c, 128 * mb)
                            oe = 512 * (c + 1)
                            esl = etile[:, _EOFF[mb] + os_ - 128 * mb:
                                        _EOFF[mb] + oe - 128 * mb]
                            st_, sp = (i == 0), (i == len(mbs) - 1)
                            nc.tensor.matmul(
                                box["ps_o"][:, os_ - 512 * c:oe - 512 * c],
                                vblk[g * NB + mb][:], esl, start=st_, stop=sp)
                        return step

                    def rowsums():
                        etile = st["etile"]
                        ps_rs = sc_tile()
                        box["ps_rs"] = ps_rs
                        for i, mb in enumerate(mbs):
                            os_ = max(512 * c, 128 * mb)
                            oe = 512 * (c + 1)
                            esl = etile[:, _EOFF[mb] + os_ - 128 * mb:
                                        _EOFF[mb] + oe - 128 * mb]
                            nc.tensor.matmul(
                                ps_rs[:, os_ - 512 * c:oe - 512 * c],
                                consts["onesb"][:], esl,
                                start=(i == 0), stop=(i == len(mbs) - 1))

                    def fin():
                        rcp = ph.tile([128, 512], F32, tag="rcp", bufs=2,
                                      name="rcp")
                        nc.vector.reciprocal_approx_fast(rcp[:], box["ps_rs"][:])
                        nc.vector.tensor_mul(
                            outtn[h][:, c * 512:(c + 1) * 512],
                            box["ps_o"][:], rcp[:])

                    return ([mk_step(i, mb) for i, mb in enumerate(mbs)]
                            + [rowsums, fin])

                def attnv_half(h, c):
                    for u in attnv_units(h, c):
                        u()

                def qproj_units(h):
                    def mk(u):
                        def step():
                            q_proj_ib(h, u)
                        return step
                    return [mk(u) for u in range(32)]

                # ---- epilogue group machinery (also used as head-7 filler)
                egroups = [(lb, c, cc) for lb in range(NB) for c in range(2)
                           for cc in range(2)]
                epi_pre = {}     # group -> held psum tile (hh 0..6 accumulated)
                epi_part = {}    # group -> sbuf bf16 partial (hh 0..6)

                def psy_mm(psy, lb, c, cc, hh, st_, sp):
                    nc.tensor.matmul(
                        psy[:],
                        outtn[hh][:, lb * 128:(lb + 1) * 128],
                        wo_t[hh][:, c * 1024 + cc * 512:
                                 c * 1024 + (cc + 1) * 512],
                        start=st_, stop=sp)

                def epi_pre_units(grp):
                    def mk(hh):
                        def step():
                            if hh == 0:
                                epi_pre[grp] = sc_tile()
                            psy_mm(epi_pre[grp], *grp, hh, hh == 0, False)
                        return step
                    return [mk(hh) for hh in range(NH - 1)]

                def epi_part_units(grp, di):
                    box = {}

                    def mk(hh):
                        def step():
                            if hh == 0:
                                box["psy"] = sc_tile()
                            psy_mm(box["psy"], *grp, hh, hh == 0,
                                   hh == NH - 2)
                        return step

                    def cp():
                        pt = ph.tile([128, 512], BF16, tag="epart", bufs=8,
                                     name="epart")
                        epi_part[grp] = pt
                        if di % 2 == 0:
                            nc.vector.tensor_copy(pt[:], box["psy"][:])
                        else:
                            nc.scalar.copy(pt[:], box["psy"][:])
                    return [mk(hh) for hh in range(NH - 1)] + [cp]

                def q_proj_ib(h, u):
                    # u in [0, 32): c-half = u // 16, ib = u % 16
                    st = qs_state[h]
                    c, ib = u // 16, u % 16
                    if u == 0:
                        st["qt"] = ph.tile([128, L], BF16, tag="qt_s", bufs=2,
                                           name="qt_s")
                        st["psqt"] = qp_tile()
                    elif u == 16:
                        st["psqt"] = qp_tile()
                    nc.tensor.matmul(
                        st["psqt"][:],
                        st["wq"][:, ib, :],
                        xt[ib][:, c * 512:(c + 1) * 512],
                        start=(ib == 0), stop=(ib == 15))
                    if u == 15:
                        # issue the c0 copy immediately (on DVE: Act is the
                        # hot queue at head start); c1's qp WAR resolves sooner
                        nc.vector.tensor_copy(st["qt"][:, 0:512], st["psqt"][:])

                # ---------------- software-pipelined head loop
                q_dma(0)
                q_dma(1)
                # Head-0 qproj fills PE while the k-rope elementwise chain
                # produces krt; kswap matmuls then run stall-free.
                for u in range(32):
                    q_proj_ib(0, u)
                # kswap = partition-halves swap of krt (pmswap permutation mm).
                # Must be emitted BEFORE q_finish(0), which reads kswap[0] —
                # Tile dependencies follow program order.
                pswk = {g: [None, None] for g in range(NKV)}
                for c in range(2):
                    for g in range(NKV):
                        pswk[g][c] = sc_tile()
                        nc.tensor.matmul(pswk[g][c][:], consts["pmswap"][:],
                                         krt[g][:, c * 512:(c + 1) * 512])
                for g in range(NKV):
                    nc.scalar.copy(kswap[g][:, 0:512], pswk[g][0][:])
                    nc.scalar.copy(kswap[g][:, 512:1024], pswk[g][1][:])
                q_finish(0)
                q_rope(0, 0)
                q_rope(0, 1)

                for h in range(NH):
                    st = qs_state[h]
                    g = h // 4
                    if h < NH - 2:
                        q_dma(h + 2)
                    if h == 4:
                        for hb in range(NH):
                            nc.sync.dma_start(wo_t[hb][:], wo[:, hb, :])
                    st["etile"] = ph.tile([128, _ETOT], BF16, tag="esc", bufs=2,
                                          name="etile")
                    etile = st["etile"]
                    qrt = st["qrt"]
                    kswap_h = st["ksw_h"]
                    # PE filler units, popped between score chunks. The attnv
                    # units sit between the two qproj c-halves so the qt-half0
                    # copy (qp slot WAR) is hidden behind attnv matmuls.
                    fillers = []
                    av = attnv_units(h - 1, 1) if h > 0 else []
                    if h < NH - 1:
                        qp_u = qproj_units(h + 1)
                        fillers += qp_u[:16] + av + qp_u[16:]
                        fillers.append(lambda hh=h + 1: q_finish(hh))
                    else:
                        # last head: fill with epilogue pre-accumulation
                        fillers += av
                        for grp in egroups[:2]:
                            fillers += epi_pre_units(grp)
                        for di, grp in enumerate(egroups[2:10]):
                            fillers += epi_part_units(grp, di)
                    fi = [0]

                    def pop_fill(n):
                        while fi[0] < len(fillers) and n > 0:
                            fillers[fi[0]]()
                            fi[0] += 1
                            n -= 1

                    rawts = {}

                    def emit_exp(mb, rawts=rawts, etile=etile, h=h):
                        # exp deferred 2 stripes so Act's bs copies (which
                        # release score PSUM slots) aren't queued behind it.
                        # Per-head state bound via defaults (late-binding!).
                        w = L - 128 * mb
                        esl = etile[:, _EOFF[mb]:_EOFF[mb] + w]
                        nc.scalar.activation(esl, rawts.pop(mb)[:], Act.Exp,
                                             bias=consts["maskb"][:, mb:mb + 1],
                                             scale=consts["alpha"][:, h:h + 1])
                        # causal triangle on the diagonal 128 cols
                        nc.gpsimd.affine_select(
                            etile[:, _EOFF[mb]:_EOFF[mb] + 128],
                            etile[:, _EOFF[mb]:_EOFF[mb] + 128],
                            pattern=[[1, 128]], compare_op=AluOp.is_ge,
                            fill=0.0, base=0, channel_multiplier=-1)

                    st["emit_exp"] = emit_exp

                    if h == NH - 1:
                        # last head: attnv(h-1,1) fillers pop during stripe 0,
                        # so h-1's deferred exps must be emitted before them
                        qs_state[h - 1]["emit_exp"](4)
                        qs_state[h - 1]["emit_exp"](5)

                    # wide and narrow stripes interleaved so the elementwise
                    # consumers aren't front-loaded; stripes 4,5 defer their
                    # exps into the next head
                    SORDER = [0, 6, 1, 3, 2, 7, 4, 5]
                    ci = 0
                    for pos in range(NB):
                        mb = SORDER[pos]
                        kb = slice(mb * 128, (mb + 1) * 128)
                        w = L - 128 * mb
                        if pos >= 2:
                            emit_exp(SORDER[pos - 2])
                        if pos == 2 and 0 < h < NH - 1:
                            qs_state[h - 1]["emit_exp"](5)
                        rawt = ph.tile([128, w], BF16, tag="raw", bufs=4,
                                       name="rawt")
                        rawts[mb] = rawt
                        for (qs, qe) in _chunks_for_stripe(mb):
                            s = qe - qs
                            # psB first: its Act copy starts the consumer
                            # chain, so issue its matmuls before psA's
                            psB = sc_tile()
                            psA = sc_tile()
                            nc.tensor.matmul(psB[:, 0:s], krt[g][64:128, kb],
                                             qrt[64:128, qs:qe])
                            nc.tensor.matmul(psB[:, s:2 * s], kswap[g][64:128, kb],
                                             qrt[64:128, qs:qe])
                            nc.tensor.matmul(psA[:, 0:s], krt[g][0:64, kb],
                                             qrt[0:64, qs:qe])
                            nc.tensor.matmul(psA[:, s:2 * s], kswap_h[0:64, kb],
                                             qrt[0:64, qs:qe])
                            bs = ph.tile([128, 512], BF16, tag="bs", bufs=4,
                                         name="bs")
                            nc.scalar.copy(bs[:, 0:2 * s], psB[:, 0:2 * s])
                            tp = ph.tile([128, 512], BF16, tag="tprod", bufs=4,
                                         name="tp")
                            nc.vector.tensor_mul(tp[:, 0:2 * s], psA[:, 0:2 * s],
                                                 bs[:, 0:2 * s])
                            rsl = rawt[:, qs - 128 * mb:qe - 128 * mb]
                            if ci % 4 == 3:
                                # all-bf16 SBUF add runs in DVE 2x mode
                                nc.vector.tensor_add(
                                    rsl, tp[:, 0:s], tp[:, s:2 * s])
                            else:
                                nc.gpsimd.tensor_add(
                                    rsl, tp[:, 0:s], tp[:, s:2 * s])
                            ci += 1
                            if ci >= 2:
                                pop_fill(3 if ci < 6 else 2)
                        if pos == 1 and 0 < h < NH - 1:
                            # previous head's deferred exps, queued past this
                            # head's widest-stripe bs copies
                            qs_state[h - 1]["emit_exp"](4)
                        elif pos == 3:
                            pop_fill(len(fillers))
                            if h < NH - 1:
                                q_rope(h + 1, 0)
                        elif pos == 4:
                            if h < NH - 1:
                                q_rope(h + 1, 1)
                        elif pos == 7:
                            attnv_half(h, 0)
                    if h == NH - 1:
                        emit_exp(4)
                        emit_exp(5)

                # ------------ epilogue: Wo projection (finish)
                attnv_half(NH - 1, 1)

                yts = {}
                for grp in egroups:
                    lb, c, cc = grp
                    if (lb, c) not in yts:
                        yts[(lb, c)] = ph.tile([128, 1024], BF16, tag="ytile",
                                               bufs=2, name="yt")
                    yt = yts[(lb, c)]
                    if grp in epi_pre:
                        psy = epi_pre[grp]
                        psy_mm(psy, lb, c, cc, NH - 1, False, True)
                    elif grp in epi_part:
                        psy = sc_tile()
                        psy_mm(psy, lb, c, cc, NH - 1, True, False)
                        nc.tensor.matmul(psy[:], consts["identb"][:],
                                         epi_part[grp][:], start=False,
                                         stop=True)
                    else:
                        psy = sc_tile()
                        for hh in range(NH):
                            psy_mm(psy, lb, c, cc, hh, hh == 0, hh == NH - 1)
                    if cc == 0:
                        nc.vector.tensor_copy(yt[:, 0:512], psy[:])
                    else:
                        nc.scalar.copy(yt[:, 512:1024], psy[:])
                        nc.sync.dma_start(
                            y[lb * 128:(lb + 1) * 128, c * 1024:(c + 1) * 1024],
                            yt[:])

    nc.compile()
    return nc


def _host_prep(x, Wq, Wk, Wv, Wo, q_param, log_scale, cos, sin, mask):
    """Build the 8 per-core input maps."""
    x = np.asarray(x, np.float32)
    Wq = np.asarray(Wq, np.float32)
    Wk = np.asarray(Wk, np.float32)
    Wv = np.asarray(Wv, np.float32)
    Wo = np.asarray(Wo, np.float32)
    cos = np.asarray(cos, np.float32)[0]      # [L, H, 64]
    sin = np.asarray(sin, np.float32)[0]
    qp = np.asarray(q_param, np.float32).reshape(H)
    ls = np.asarray(log_scale, np.float32).reshape(H)
    mask = np.asarray(mask)

    p64 = np.arange(128) % 64

    PM = np.zeros((128, 128), np.float32)
    for dp in range(128):
        base, r = (dp // 64) * 64, dp % 64
        if r < 32:
            PM[base + r + 32, dp] = -1.0
        else:
            PM[base + r - 32, dp] = 1.0
    SW = np.zeros((128, 128), np.float32)
    for dp in range(128):
        SW[(dp + 64) % 128, dp] = 1.0
    PM = PM.astype(ml_dtypes.bfloat16)
    SW = SW.astype(ml_dtypes.bfloat16)
    ONES = np.ones((128, 128), ml_dtypes.bfloat16)
    IDENT = np.eye(128, dtype=ml_dtypes.bfloat16)

    in_maps = []
    for core in range(8):
        b, g2 = core // 2, core % 2
        heads = list(range(g2 * NH, (g2 + 1) * NH))
        kvs = list(range(g2 * NKV, (g2 + 1) * NKV))

        xh = x[b].astype(ml_dtypes.bfloat16)

        wq_c = Wq[:, g2 * NH * 128:(g2 + 1) * NH * 128]
        wk_c = Wk[:, g2 * NKV * 128:(g2 + 1) * NKV * 128]
        wv_c = Wv[:, g2 * NKV * 128:(g2 + 1) * NKV * 128]
        wo_c = Wo[g2 * NH * 128:(g2 + 1) * NH * 128, :]

        # wq: [128(part=K slice), NH, 16(ib), 128(dq)]
        wq_p = wq_c.reshape(16, 128, NH, 128).transpose(1, 2, 0, 3)
        wq_p = np.ascontiguousarray(wq_p).astype(ml_dtypes.bfloat16)
        wk_p = wk_c.reshape(16, 128, NKV, 128).transpose(1, 2, 0, 3)
        wk_p = np.ascontiguousarray(wk_p).astype(ml_dtypes.bfloat16)
        wv_p = wv_c.reshape(16, 128, NKV, 128).transpose(1, 2, 0, 3)
        wv_p = np.ascontiguousarray(wv_p).astype(ml_dtypes.bfloat16)
        wo_p = wo_c.reshape(NH, 128, D).transpose(1, 0, 2)
        wo_p = np.ascontiguousarray(wo_p).astype(ml_dtypes.bfloat16)

        cosq_p = np.ascontiguousarray(
            cos[:, heads, :][:, :, p64].transpose(2, 1, 0)).astype(ml_dtypes.bfloat16)
        sinq_p = np.ascontiguousarray(
            sin[:, heads, :][:, :, p64].transpose(2, 1, 0)).astype(ml_dtypes.bfloat16)
        cosk_p = np.ascontiguousarray(
            cos[:, kvs, :][:, :, p64].transpose(2, 1, 0)).astype(ml_dtypes.bfloat16)
        sink_p = np.ascontiguousarray(
            sin[:, kvs, :][:, :, p64].transpose(2, 1, 0)).astype(ml_dtypes.bfloat16)

        mb = np.where(mask[b].reshape(NB, 128).T.astype(bool), 0.0, -1e9)
        mb = mb.astype(np.float32)

        cpr = np.tile((-2.0 * np.tanh(qp[heads]))[None, :], (128, 1))
        alp = np.tile((np.exp(ls[heads]) / HD)[None, :], (128, 1))

        in_maps.append({
            "xh": xh,
            "wq": wq_p, "wk": wk_p, "wv": wv_p, "wo": wo_p,
            "cosq": cosq_p, "sinq": sinq_p, "cosk": cosk_p, "sink": sink_p,
            "maskb": mb, "cprime": cpr.astype(np.float32),
            "alpha": alp.astype(np.float32),
            "pmrot": PM, "pmswap": SW, "onesb": ONES, "identb": IDENT,
        })
    return in_maps


def kernel(**inputs):
    if "nc" not in _CACHED:
        _CACHED["nc"] = build_program()
    nc = _CACHED["nc"]
    in_maps = _host_prep(**inputs)
    res = run_bass_kernel_spmd(nc, in_maps, list(range(8))).results
    out = np.empty((B, L, D), np.float32)
    for b in range(B):
        out[b] = (res[2 * b]["y"].astype(np.float32)
                  + res[2 * b + 1]["y"].astype(np.float32))
    return out


# revision 103
# speedup vs baseline: 1.0133x; 1.0047x over previous
"""BivectorRotarySelfAttention TRN2 kernel (bf16 pipeline).

Sharding: 8 cores = 4 batches x 2 head-halves. Each core computes one batch's
attention for 8 heads (2 kv heads) and a partial output projection; host sums
the two head-half partials per batch (bf16 partials, f32 sum).

Per-core dataflow (transposed layouts: features in partitions, seq in free):
  xT[ib]  = dma_transpose(x_bf16)                       16 x [128, L] bf16
  qT/kT/vT = W.T @ xT   (bf16 matmuls, PSUM f32, copied out as bf16)
  rope (per 512-half): psr = pmrot@qt (PE); t1 = psr*sin (DVE);
        t2 = qt*cos (Pool); qrt = t1+t2 (Pool)
  scores S^T[m,q]: 4 K=64 bf16 matmuls per 256-col chunk:
        psA = [S0 | c'*C0] (krt/kswap_h), psB = [S1 | C1] (krt/kswap)
        bs  = copy(psB) (Act), tp = psA*bs (DVE),
        raw = tp[:s] + tp[s:] (Pool), E = exp(alpha*raw + maskbias) (Act)
  causal: affine_select on diagonal blocks (Pool)
  outT[d,q] = vblk.T @ E; rowsums via ones-matmul; outtn = ps_o * rcp (DVE)
  y[l,:] = sum_h outtn_h.T @ Wo_h  (bf16 matmuls, bf16 out, f32 host sum)

Scheduling: engines execute in scheduled (program-priority) order, so the
head loop is software-pipelined by construction: head h's stripe loop emits
next head's q-projection/rope and the previous head's attnv c=1 as filler
units between score chunks; head 7 is filled with epilogue pre-accumulation
(heads 0..6 of the first Wo groups, parked to SBUF and finished via an
identity-matmul accumulate). DMA: weight copies first, ONE xbar switch,
then all 16 x-transposes back-to-back (copy<->transpose switches ~2.2us).
NOTE: Tile dependencies follow program order — a tile must be written
before any reader is emitted (head-0 kswap ordering).
"""
import sys
if '/opt/trn_rl_repo' not in sys.path:
    sys.path.insert(0, '/opt/trn_rl_repo')

import numpy as np
import ml_dtypes

import concourse.bass as bass
import concourse.mybir as mybir
import concourse.tile as tile
from concourse import bacc
from concourse.bass_utils import run_bass_kernel_spmd

F32 = mybir.dt.float32
BF16 = mybir.dt.bfloat16

B, L, D, H, HKV = 4, 1024, 2048, 16, 4
HD = D // H            # 128
HD2 = HD // 2          # 64
NH = 8                 # heads per core
NKV = 2                # kv heads per core
NB = L // 128          # 8 blocks of 128
AluOp = mybir.AluOpType
Act = mybir.ActivationFunctionType

_CACHED = {}


def _chunks_for_stripe(mb):
    """Q-column chunks [(qs, qe)] covering [128*mb, 1024), split at 256-multiples."""
    q0 = 128 * mb
    out = []
    while q0 < L:
        qe = min(L, (q0 // 256 + 1) * 256)
        out.append((q0, qe))
        q0 = qe
    return out


# packed E-tile column offsets: region for stripe mb starts at _EOFF[mb]
_EOFF = [0]
for _mb in range(NB):
    _EOFF.append(_EOFF[-1] + (L - 128 * _mb))
_ETOT = _EOFF[NB]          # 4608


def build_program():
    nc = bacc.Bacc("TRN2", target_bir_lowering=False, debug=False)

    # ---- dram params (per-core shapes) ----
    xh = nc.declare_dram_parameter("xh", [L, D], BF16, isOutput=False)
    wq = nc.declare_dram_parameter("wq", [128, NH, 16, 128], BF16, isOutput=False)
    wk = nc.declare_dram_parameter("wk", [128, NKV, 16, 128], BF16, isOutput=False)
    wv = nc.declare_dram_parameter("wv", [128, NKV, 16, 128], BF16, isOutput=False)
    wo = nc.declare_dram_parameter("wo", [128, NH, D], BF16, isOutput=False)
    cosq = nc.declare_dram_parameter("cosq", [128, NH, L], BF16, isOutput=False)
    sinq = nc.declare_dram_parameter("sinq", [128, NH, L], BF16, isOutput=False)
    cosk = nc.declare_dram_parameter("cosk", [128, NKV, L], BF16, isOutput=False)
    sink = nc.declare_dram_parameter("sink", [128, NKV, L], BF16, isOutput=False)
    maskb = nc.declare_dram_parameter("maskb", [128, NB], F32, isOutput=False)
    cprime = nc.declare_dram_parameter("cprime", [128, NH], F32, isOutput=False)
    alpha = nc.declare_dram_parameter("alpha", [128, NH], F32, isOutput=False)
    pmrot = nc.declare_dram_parameter("pmrot", [128, 128], BF16, isOutput=False)
    pmswap = nc.declare_dram_parameter("pmswap", [128, 128], BF16, isOutput=False)
    onesb = nc.declare_dram_parameter("onesb", [128, 128], BF16, isOutput=False)
    identb = nc.declare_dram_parameter("identb", [128, 128], BF16, isOutput=False)
    y = nc.declare_dram_parameter("y", [L, D], BF16, isOutput=True)

    with tile.TileContext(nc) as tc:
        with (
            tc.tile_pool(name="persist", bufs=1) as pp,
            tc.tile_pool(name="psum", bufs=1, space="PSUM") as psp,
        ):
            # PSUM tags: "qp" [128,512] bufs=1 (1 bank) for q projections,
            # "sc" [128,512] bufs=7 (7 banks) for scores/attnv/vT/epilogue.
            def qp_tile():
                return psp.tile([128, 512], F32, tag="qp", bufs=1, name="qp_t")

            def sc_tile(w=512, dt_=F32):
                return psp.tile([128, w], dt_, tag="sc", bufs=7, name="sc_t")

            # --- DMA order: weight copies first, ONE xbar switch, then all
            # 16 transposes back-to-back (copy<->transpose switches cost ~2.2us)
            wk_t = pp.tile([128, NKV, 16, 128], BF16, tag="wk", name="wk_t")
            wv_t = pp.tile([128, NKV, 16, 128], BF16, tag="wv", name="wv_t")
            xt = [pp.tile([128, L], BF16, tag=f"xt{ib}", name=f"xt{ib}")
                  for ib in range(16)]
            nc.sync.dma_start(wk_t[:, 0], wk[:, 0])
            nc.sync.dma_start(wv_t[:, 0], wv[:, 0])
            for ib in range(16):
                nc.sync.dma_start_transpose(xt[ib][:], xh[:, ib * 128:(ib + 1) * 128])
            nc.sync.dma_start(wk_t[:, 1], wk[:, 1])
            nc.sync.dma_start(wv_t[:, 1], wv[:, 1])

            # small consts + k tables + head-0 tables next
            consts = {}
            for nm, src, dt_ in [("pmrot", pmrot, BF16), ("pmswap", pmswap, BF16),
                                 ("onesb", onesb, BF16), ("identb", identb, BF16),
                                 ("maskb", maskb, F32), ("cprime", cprime, F32),
                                 ("alpha", alpha, F32)]:
                t = pp.tile(list(src.shape), dt_, tag=nm, name=nm)
                nc.sync.dma_start(t[:], src[:])
                consts[nm] = t
            csl = pp.tile([128, NKV, L], BF16, tag="cosk", name="csl")
            snl = pp.tile([128, NKV, L], BF16, tag="sink", name="snl")
            nc.sync.dma_start(csl[:], cosk[:])
            nc.sync.dma_start(snl[:], sink[:])

            krt = [pp.tile([128, L], BF16, tag=f"krt{g}", name=f"krt{g}")
                   for g in range(NKV)]
            kswap = [pp.tile([128, L], BF16, tag=f"ksw{g}", name=f"ksw{g}")
                     for g in range(NKV)]
            vblk = [pp.tile([128, 128], BF16, tag=f"vb{i}", name=f"vb{i}")
                    for i in range(NKV * NB)]
            outtn = [pp.tile([128, L], BF16, tag=f"ot{h}", name=f"ot{h}")
                     for h in range(NH)]
            wo_t = [pp.tile([128, D], BF16, tag=f"wo{hb}", name=f"wo{hb}")
                    for hb in range(NH)]

            # ---------------- prologue: k/v proj pipelined via sc psum slots
            with (tc.tile_pool(name="pro", bufs=1) as ppro,
                  tc.tile_pool(name="hl", bufs=1) as ph):
                kt_s, vt_s = [], []
                projs = []
                for g in range(NKV):
                    projs.append((wk_t, g, kt_s, f"kt{g}"))
                    projs.append((wv_t, g, vt_s, f"vt{g}"))
                for w_t, g, outl, tg in projs:
                    pj = [sc_tile(), sc_tile()]
                    for ib in range(16):
                        for c in range(2):
                            nc.tensor.matmul(
                                pj[c][:],
                                w_t[:, g, ib, :],
                                xt[ib][:, c * 512:(c + 1) * 512],
                                start=(ib == 0), stop=(ib == 15))
                    ot = ppro.tile([128, L], BF16, tag=tg, name="projout")
                    if tg.startswith("kt"):
                        nc.scalar.copy(ot[:, 0:512], pj[0][:])
                        nc.scalar.copy(ot[:, 512:1024], pj[1][:])
                    else:
                        nc.vector.tensor_copy(ot[:, 0:512], pj[0][:])
                        nc.vector.tensor_copy(ot[:, 512:1024], pj[1][:])
                    outl.append(ot)

                # v transposes (fill PE while k copies/ropes progress)
                for g in range(NKV):
                    for mb in range(NB):
                        pv = sc_tile(128, BF16)
                        nc.tensor.transpose(pv[:], vt_s[g][:, mb * 128:(mb + 1) * 128],
                                            consts["identb"][:])
                        if mb % 2 == 0:
                            nc.vector.tensor_copy(vblk[g * NB + mb][:], pv[:])
                        else:
                            nc.scalar.copy(vblk[g * NB + mb][:], pv[:])

                # k rotate matmuls
                psrk = {}
                for g in range(NKV):
                    psrk[g] = [sc_tile(), sc_tile()]
                    for c in range(2):
                        nc.tensor.matmul(psrk[g][c][:], consts["pmrot"][:],
                                         kt_s[g][:, c * 512:(c + 1) * 512])
                # c0 halves for both groups first, so the pswk c0 matmuls
                # (emitted in the same order) don't wait on c1's Pool chain
                for c in range(2):
                    for g in range(NKV):
                        cs = slice(c * 512, (c + 1) * 512)
                        t1k = ppro.tile([128, 512], BF16, tag="rtmp", bufs=2,
                                        name="t1k")
                        t2k = ppro.tile([128, 512], BF16, tag="rtmp", bufs=2,
                                        name="t2k")
                        nc.vector.tensor_mul(t1k[:], psrk[g][c][:], snl[:, g, cs])
                        nc.vector.tensor_mul(t2k[:], kt_s[g][:, cs], csl[:, g, cs])
                        nc.vector.tensor_add(krt[g][:, cs], t1k[:], t2k[:])

                # ---------------- head-pipeline helpers
                qs_state = {}

                def q_dma(h):
                    st = {}
                    st["wq"] = ph.tile([128, 16, 128], BF16, tag="wq_h", bufs=2,
                                       name="wq_t")
                    nc.sync.dma_start(st["wq"][:], wq[:, h, :, :])
                    st["cq"] = ph.tile([128, L], BF16, tag="cq", bufs=2, name="cq")
                    st["sq"] = ph.tile([128, L], BF16, tag="sq", bufs=2, name="sq")
                    nc.sync.dma_start(st["cq"][:], cosq[:, h, :])
                    nc.sync.dma_start(st["sq"][:], sinq[:, h, :])
                    qs_state[h] = st

                def q_finish(h):
                    st = qs_state[h]
                    nc.scalar.copy(st["qt"][:, 512:1024], st["psqt"][:])
                    st["ksw_h"] = ph.tile([128, L], BF16, tag="ksw_h", bufs=2,
                                          name="kswap_h")
                    nc.vector.tensor_scalar_mul(
                        st["ksw_h"][:], kswap[h // 4][:],
                        consts["cprime"][:, h:h + 1])

                def q_rope(h, c):
                    st = qs_state[h]
                    if c == 0:
                        st["qrt"] = ph.tile([128, L], BF16, tag="qrt", bufs=2,
                                            name="qrt")
                    cs = slice(c * 512, (c + 1) * 512)
                    psr = sc_tile()
                    nc.tensor.matmul(psr[:], consts["pmrot"][:], st["qt"][:, cs])
                    t1 = ph.tile([128, 512], BF16, tag="qtmp", bufs=2, name="t1")
                    t2 = ph.tile([128, 512], BF16, tag="qtmp", bufs=2, name="t2")
                    nc.vector.tensor_mul(t1[:], psr[:], st["sq"][:, cs])
                    nc.vector.tensor_mul(t2[:], st["qt"][:, cs], st["cq"][:, cs])
                    nc.vector.tensor_add(st["qrt"][:, cs], t1[:], t2[:])

                def attnv_units(h, c):
                    """Closures: accumulation steps + rowsums + normalize."""
                    st = qs_state[h]
                    g = h // 4
                    mbs = [mb for mb in range(NB) if 128 * mb < 512 * (c + 1)]
                    box = {}

                    def mk_step(i, mb):
                        def step():
                            if i == 0:
                                box["ps_o"] = sc_tile()
                            etile = st["etile"]
                            os_ = max(512 * c, 128 * mb)
                            oe = 512 * (c + 1)
                            esl = etile[:, _EOFF[mb] + os_ - 128 * mb:
                                        _EOFF[mb] + oe - 128 * mb]
                            st_, sp = (i == 0), (i == len(mbs) - 1)
                            nc.tensor.matmul(
                                box["ps_o"][:, os_ - 512 * c:oe - 512 * c],
                                vblk[g * NB + mb][:], esl, start=st_, stop=sp)
                        return step

                    def rowsums():
                        etile = st["etile"]
                        ps_rs = sc_tile()
                        box["ps_rs"] = ps_rs
                        for i, mb in enumerate(mbs):
                            os_ = max(512 * c, 128 * mb)
                            oe = 512 * (c + 1)
                            esl = etile[:, _EOFF[mb] + os_ - 128 * mb:
                                        _EOFF[mb] + oe - 128 * mb]
                            nc.tensor.matmul(
                                ps_rs[:, os_ - 512 * c:oe - 512 * c],
                                consts["onesb"][:], esl,
                                start=(i == 0), stop=(i == len(mbs) - 1))

                    def fin():
                        rcp = ph.tile([128, 512], F32, tag="rcp", bufs=2,
                                      name="rcp")
                        nc.vector.reciprocal_approx_fast(rcp[:], box["ps_rs"][:])
                        nc.vector.tensor_mul(
                            outtn[h][:, c * 512:(c + 1) * 512],
                            box["ps_o"][:], rcp[:])

                    return ([mk_step(i, mb) for i, mb in enumerate(mbs)]
                            + [rowsums, fin])

                def attnv_half(h, c):
                    for u in attnv_units(h, c):
                        u()

                def qproj_units(h):
                    def mk(u):
                        def step():
                            q_proj_ib(h, u)
                        return step
                    return [mk(u) for u in range(32)]

                # ---- epilogue group machinery (also used as head-7 filler)
                egroups = [(lb, c, cc) for lb in range(NB) for c in range(2)
                           for cc in range(2)]
                epi_pre = {}     # group -> held psum tile (hh 0..6 accumulated)
                epi_part = {}    # group -> sbuf bf16 partial (hh 0..6)

                def psy_mm(psy, lb, c, cc, hh, st_, sp):
                    nc.tensor.matmul(
                        psy[:],
                        outtn[hh][:, lb * 128:(lb + 1) * 128],
                        wo_t[hh][:, c * 1024 + cc * 512:
                                 c * 1024 + (cc + 1) * 512],
                        start=st_, stop=sp)

                def epi_pre_units(grp):
                    def mk(hh):
                        def step():
                            if hh == 0:
                                epi_pre[grp] = sc_tile()
                            psy_mm(epi_pre[grp], *grp, hh, hh == 0, False)
                        return step
                    return [mk(hh) for hh in range(NH - 1)]

                def epi_part_units(grp, di):
                    box = {}

                    def mk(hh):
                        def step():
                            if hh == 0:
                                box["psy"] = sc_tile()
                            psy_mm(box["psy"], *grp, hh, hh == 0,
                                   hh == NH - 2)
                        return step

                    def cp():
                        pt = ph.tile([128, 512], BF16, tag="epart", bufs=8,
                                     name="epart")
                        epi_part[grp] = pt
                        if di % 2 == 0:
                            nc.vector.tensor_copy(pt[:], box["psy"][:])
                        else:
                            nc.scalar.copy(pt[:], box["psy"][:])
                    return [mk(hh) for hh in range(NH - 1)] + [cp]

                def q_proj_ib(h, u):
                    # u in [0, 32): c-half = u // 16, ib = u % 16
                    st = qs_state[h]
                    c, ib = u // 16, u % 16
                    if u == 0:
                        st["qt"] = ph.tile([128, L], BF16, tag="qt_s", bufs=2,
                                           name="qt_s")
                        st["psqt"] = qp_tile()
                    elif u == 16:
                        st["psqt"] = qp_tile()
                    nc.tensor.matmul(
                        st["psqt"][:],
                        st["wq"][:, ib, :],
                        xt[ib][:, c * 512:(c + 1) * 512],
                        start=(ib == 0), stop=(ib == 15))
                    if u == 15:
                        # issue the c0 copy immediately (on DVE: Act is the
                        # hot queue at head start); c1's qp WAR resolves sooner
                        nc.vector.tensor_copy(st["qt"][:, 0:512], st["psqt"][:])

                # ---------------- software-pipelined head loop
                q_dma(0)
                q_dma(1)
                # Head-0 qproj fills PE while the k-rope elementwise chain
                # produces krt; kswap matmuls then run stall-free.
                for u in range(32):
                    q_proj_ib(0, u)
                # kswap = partition-halves swap of krt (pmswap permutation mm).
                # Must be emitted BEFORE q_finish(0), which reads kswap[0] —
                # Tile dependencies follow program order.
                pswk = {g: [None, None] for g in range(NKV)}
                for c in range(2):
                    for g in range(NKV):
                        pswk[g][c] = sc_tile()
                        nc.tensor.matmul(pswk[g][c][:], consts["pmswap"][:],
                                         krt[g][:, c * 512:(c + 1) * 512])
                for g in range(NKV):
                    nc.scalar.copy(kswap[g][:, 0:512], pswk[g][0][:])
                    nc.scalar.copy(kswap[g][:, 512:1024], pswk[g][1][:])
                q_finish(0)
                q_rope(0, 0)
                q_rope(0, 1)

                for h in range(NH):
                    st = qs_state[h]
                    g = h // 4
                    if h < NH - 2:
                        q_dma(h + 2)
                    if h == 4:
                        for hb in range(NH):
                            nc.sync.dma_start(wo_t[hb][:], wo[:, hb, :])
                    st["etile"] = ph.tile([128, _ETOT], BF16, tag="esc", bufs=2,
                                          name="etile")
                    etile = st["etile"]
                    qrt = st["qrt"]
                    kswap_h = st["ksw_h"]
                    # PE filler units, popped between score chunks. The attnv
                    # units sit between the two qproj c-halves so the qt-half0
                    # copy (qp slot WAR) is hidden behind attnv matmuls.
                    fillers = []
                    av = attnv_units(h - 1, 1) if h > 0 else []
                    if h < NH - 1:
                        qp_u = qproj_units(h + 1)
                        fillers += qp_u[:16] + av + qp_u[16:]
                        fillers.append(lambda hh=h + 1: q_finish(hh))
                    else:
                        # last head: fill with epilogue pre-accumulation
                        fillers += av
                        for grp in egroups[:2]:
                            fillers += epi_pre_units(grp)
                        for di, grp in enumerate(egroups[2:10]):
                            fillers += epi_part_units(grp, di)
                    fi = [0]

                    def pop_fill(n):
                        while fi[0] < len(fillers) and n > 0:
                            fillers[fi[0]]()
                            fi[0] += 1
                            n -= 1

                    rawts = {}

                    def emit_exp(mb, rawts=rawts, etile=etile, h=h):
                        # exp deferred 2 stripes so Act's bs copies (which
                        # release score PSUM slots) aren't queued behind it.
                        # Per-head state bound via defaults (late-binding!).
                        w = L - 128 * mb
                        esl = etile[:, _EOFF[mb]:_EOFF[mb] + w]
                        nc.scalar.activation(esl, rawts.pop(mb)[:], Act.Exp,
                                             bias=consts["maskb"][:, mb:mb + 1],
                                             scale=consts["alpha"][:, h:h + 1])
                        # causal triangle on the diagonal 128 cols
                        nc.gpsimd.affine_select(
                            etile[:, _EOFF[mb]:_EOFF[mb] + 128],
                            etile[:, _EOFF[mb]:_EOFF[mb] + 128],
                            pattern=[[1, 128]], compare_op=AluOp.is_ge,
                            fill=0.0, base=0, channel_multiplier=-1)

                    st["emit_exp"] = emit_exp

                    if h == NH - 1:
                        # last head: attnv(h-1,1) fillers pop during stripe 0,
                        # so h-1's deferred exps must be emitted before them
                        qs_state[h - 1]["emit_exp"](4)
                        qs_state[h - 1]["emit_exp"](5)

                    # wide and narrow stripes interleaved so the elementwise
                    # consumers aren't front-loaded; stripes 4,5 defer their
                    # exps into the next head
                    SORDER = [0, 6, 1, 3, 2, 7, 4, 5]
                    ci = 0
                    for pos in range(NB):
                        mb = SORDER[pos]
                        kb = slice(mb * 128, (mb + 1) * 128)
                        w = L - 128 * mb
                        if pos >= 2:
                            emit_exp(SORDER[pos - 2])
                        if pos == 2 and 0 < h < NH - 1:
                            qs_state[h - 1]["emit_exp"](5)
                        rawt = ph.tile([128, w], BF16, tag="raw", bufs=4,
                                       name="rawt")
                        rawts[mb] = rawt
                        for (qs, qe) in _chunks_for_stripe(mb):
                            s = qe - qs
                            # psB first: its Act copy starts the consumer
                            # chain, so issue its matmuls before psA's
                            psB = sc_tile()
                            psA = sc_tile()
                            nc.tensor.matmul(psB[:, 0:s], krt[g][64:128, kb],
                                             qrt[64:128, qs:qe])
                            nc.tensor.matmul(psB[:, s:2 * s], kswap[g][64:128, kb],
                                             qrt[64:128, qs:qe])
                            nc.tensor.matmul(psA[:, 0:s], krt[g][0:64, kb],
                                             qrt[0:64, qs:qe])
                            nc.tensor.matmul(psA[:, s:2 * s], kswap_h[0:64, kb],
                                             qrt[0:64, qs:qe])
                            bs = ph.tile([128, 512], BF16, tag="bs", bufs=4,
                                         name="bs")
                            nc.scalar.copy(bs[:, 0:2 * s], psB[:, 0:2 * s])
                            tp = ph.tile([128, 512], BF16, tag="tprod", bufs=4,
                                         name="tp")
                            nc.vector.tensor_mul(tp[:, 0:2 * s], psA[:, 0:2 * s],
                                                 bs[:, 0:2 * s])
                            rsl = rawt[:, qs - 128 * mb:qe - 128 * mb]
                            if ci % 4 == 3:
                                # all-bf16 SBUF add runs in DVE 2x mode
                                nc.vector.tensor_add(
                                    rsl, tp[:, 0:s], tp[:, s:2 * s])
                            else:
                                nc.gpsimd.tensor_add(
                                    rsl, tp[:, 0:s], tp[:, s:2 * s])
                            ci += 1
                            if ci >= 2:
                                pop_fill(3 if ci < 6 else 2)
                        if pos == 1 and 0 < h < NH - 1:
                            # previous head's deferred exps, queued past this
                            # head's widest-stripe bs copies
                            qs_state[h - 1]["emit_exp"](4)
                        elif pos == 3:
                            pop_fill(len(fillers))
                            if h < NH - 1:
                                q_rope(h + 1, 0)
                        elif pos == 4:
                            if h < NH - 1:
                                q_rope(h + 1, 1)
                        elif pos == 7:
                            attnv_half(h, 0)
                    if h == NH - 1:
                        emit_exp(4)
                        emit_exp(5)

                # ------------ epilogue: Wo projection (finish)
                attnv_half(NH - 1, 1)

                yts = {}
                for grp in egroups:
                    lb, c, cc = grp
                    if (lb, c) not in yts:
                        yts[(lb, c)] = ph.tile([128, 1024], BF16, tag="ytile",
                                               bufs=2, name="yt")
                    yt = yts[(lb, c)]
                    if grp in epi_pre:
                        psy = epi_pre[grp]
                        psy_mm(psy, lb, c, cc, NH - 1, False, True)
                    elif grp in epi_part:
                        psy = sc_tile()
                        psy_mm(psy, lb, c, cc, NH - 1, True, False)
                        nc.tensor.matmul(psy[:], consts["identb"][:],
                                         epi_part[grp][:], start=False,
                                         stop=True)
                    else:
                        psy = sc_tile()
                        for hh in range(NH):
                            psy_mm(psy, lb, c, cc, hh, hh == 0, hh == NH - 1)
                    if cc == 0:
                        nc.vector.tensor_copy(yt[:, 0:512], psy[:])
                    else:
                        nc.scalar.copy(yt[:, 512:1024], psy[:])
                        nc.sync.dma_start(
                            y[lb * 128:(lb + 1) * 128, c * 1024:(c + 1) * 1024],
                            yt[:])

    nc.compile()
    return nc


def _host_prep(x, Wq, Wk, Wv, Wo, q_param, log_scale, cos, sin, mask):
    """Build the 8 per-core input maps."""
    x = np.asarray(x, np.float32)
    Wq = np.asarray(Wq, np.float32)
    Wk = np.asarray(Wk, np.float32)
    Wv = np.asarray(Wv, np.float32)
    Wo = np.asarray(Wo, np.float32)
    cos = np.asarray(cos, np.float32)[0]      # [L, H, 64]
    sin = np.asarray(sin, np.float32)[0]
    qp = np.asarray(q_param, np.float32).reshape(H)
    ls = np.asarray(log_scale, np.float32).reshape(H)
    mask = np.asarray(mask)

    p64 = np.arange(128) % 64

    PM = np.zeros((128, 128), np.float32)
    for dp in range(128):
        base, r = (dp // 64) * 64, dp % 64
        if r < 32:
            PM[base + r + 32, dp] = -1.0
        else:
            PM[base + r - 32, dp] = 1.0
    SW = np.zeros((128, 128), np.float32)
    for dp in range(128):
        SW[(dp + 64) % 128, dp] = 1.0
    PM = PM.astype(ml_dtypes.bfloat16)
    SW = SW.astype(ml_dtypes.bfloat16)
    ONES = np.ones((128, 128), ml_dtypes.bfloat16)
    IDENT = np.eye(128, dtype=ml_dtypes.bfloat16)

    in_maps = []
    for core in range(8):
        b, g2 = core // 2, core % 2
        heads = list(range(g2 * NH, (g2 + 1) * NH))
        kvs = list(range(g2 * NKV, (g2 + 1) * NKV))

        xh = x[b].astype(ml_dtypes.bfloat16)

        wq_c = Wq[:, g2 * NH * 128:(g2 + 1) * NH * 128]
        wk_c = Wk[:, g2 * NKV * 128:(g2 + 1) * NKV * 128]
        wv_c = Wv[:, g2 * NKV * 128:(g2 + 1) * NKV * 128]
        wo_c = Wo[g2 * NH * 128:(g2 + 1) * NH * 128, :]

        # wq: [128(part=K slice), NH, 16(ib), 128(dq)]
        wq_p = wq_c.reshape(16, 128, NH, 128).transpose(1, 2, 0, 3)
        wq_p = np.ascontiguousarray(wq_p).astype(ml_dtypes.bfloat16)
        wk_p = wk_c.reshape(16, 128, NKV, 128).transpose(1, 2, 0, 3)
        wk_p = np.ascontiguousarray(wk_p).astype(ml_dtypes.bfloat16)
        wv_p = wv_c.reshape(16, 128, NKV, 128).transpose(1, 2, 0, 3)
        wv_p = np.ascontiguousarray(wv_p).astype(ml_dtypes.bfloat16)
        wo_p = wo_c.reshape(NH, 128, D).transpose(1, 0, 2)
        wo_p = np.ascontiguousarray(wo_p).astype(ml_dtypes.bfloat16)

        cosq_p = np.ascontiguousarray(
            cos[:, heads, :][:, :, p64].transpose(2, 1, 0)).astype(ml_dtypes.bfloat16)
        sinq_p = np.ascontiguousarray(
            sin[:, heads, :][:, :, p64].transpose(2, 1, 0)).astype(ml_dtypes.bfloat16)
        cosk_p = np.ascontiguousarray(
            cos[:, kvs, :][:, :, p64].transpose(2, 1, 0)).astype(ml_dtypes.bfloat16)
        sink_p = np.ascontiguousarray(
            sin[:, kvs, :][:, :, p64].transpose(2, 1, 0)).astype(ml_dtypes.bfloat16)

        mb = np.where(mask[b].reshape(NB, 128).T.astype(bool), 0.0, -1e9)
        mb = mb.astype(np.float32)

        cpr = np.tile((-2.0 * np.tanh(qp[heads]))[None, :], (128, 1))
        alp = np.tile((np.exp(ls[heads]) / HD)[None, :], (128, 1))

        in_maps.append({
            "xh": xh,
            "wq": wq_p, "wk": wk_p, "wv": wv_p, "wo": wo_p,
            "cosq": cosq_p, "sinq": sinq_p, "cosk": cosk_p, "sink": sink_p,
            "maskb": mb, "cprime": cpr.astype(np.float32),
            "alpha": alp.astype(np.float32),
            "pmrot": PM, "pmswap": SW, "onesb": ONES, "identb": IDENT,
        })
    return in_maps


def kernel(**inputs):
    if "nc" not in _CACHED:
        _CACHED["nc"] = build_program()
    nc = _CACHED["nc"]
    in_maps = _host_prep(**inputs)
    res = run_bass_kernel_spmd(nc, in_maps, list(range(8))).results
    out = np.empty((B, L, D), np.float32)
    for b in range(B):
        out[b] = (res[2 * b]["y"].astype(np.float32)
                  + res[2 * b + 1]["y"].astype(np.float32))
    return out
